# revision 15
# baseline (speedup 1.0000x reference)
"""ConceptBank retrieval-KNN kernel for 8 Trainium2 NeuronCores.

Reference computation (see problem):
  queries = LayerNorm(registers) @ Wq + bq            [B,R,128]
  row/col scores vs 500 row/col keys (product keys)   [B,R,500] each
  top-4 over the 250000 outer-sum scores              (exact via top-4 x top-4 candidates)
  gather concepts[topk_idx]                           [B,R,4,768]
  query_projected = queries @ Wqc + bqc               [B,R,768]

Sharding: data-parallel over the 256 (B*R) tokens -> 32 tokens per core.
All weights/keys and the 768MB concepts table are replicated per core;
the gather reads only 128 rows per core so replication is the fastest
layout (no collectives on the critical path).
"""

import numpy as np

B, R, D = 4, 64, 768
KD = 128            # key_dim
H = KD // 2         # 64, half key dim
NK = 500            # n row/col keys
NCPT = NK * NK      # 250000 concepts
TOPK = 4
NCORES = 8
NTOK = B * R        # 256
T = NTOK // NCORES  # 32 tokens per core

_CACHE = {}


def _build_nc():
    import concourse.bacc as bacc
    import concourse.mybir as mybir
    import concourse.tile as tile
    from concourse.masks import make_identity

    f32 = mybir.dt.float32
    i32 = mybir.dt.int32
    u32 = mybir.dt.uint32

    # Bacc (not plain Bass): its compile() pass splits multi-semaphore waits
    # into event-semaphore instructions, which walrus codegen requires.
    nc = bacc.Bacc(None)

    # ---- DRAM I/O ----
    x_h = nc.dram_tensor("x_local", [T, D], f32, kind="ExternalInput")
    rk_h = nc.dram_tensor("row_keys", [NK, H], f32, kind="ExternalInput")
    ck_h = nc.dram_tensor("col_keys", [NK, H], f32, kind="ExternalInput")
    cpt_h = nc.dram_tensor("concepts", [NCPT, D], f32, kind="ExternalInput")
    gam_h = nc.dram_tensor("ln_gamma", [1, D], f32, kind="ExternalInput")
    bet_h = nc.dram_tensor("ln_beta", [1, D], f32, kind="ExternalInput")
    wq_h = nc.dram_tensor("Wq", [D, KD], f32, kind="ExternalInput")
    bq_h = nc.dram_tensor("bq", [1, KD], f32, kind="ExternalInput")
    wqc_h = nc.dram_tensor("Wqc", [KD, D], f32, kind="ExternalInput")
    bqc_h = nc.dram_tensor("bqc", [1, D], f32, kind="ExternalInput")

    oc_h = nc.dram_tensor("out_concepts", [T * TOPK, D], f32, kind="ExternalOutput")
    oi_h = nc.dram_tensor("out_idx", [T, TOPK], i32, kind="ExternalOutput")
    os_h = nc.dram_tensor("out_scores", [T, TOPK], f32, kind="ExternalOutput")
    oq_h = nc.dram_tensor("out_qproj", [T, D], f32, kind="ExternalOutput")

    DCH = D // 128           # 6 chunks of the D dim
    KCH = 4                  # key rows chunked 4 x 125
    KP = NK // KCH           # 125

    with tile.TileContext(nc) as tc:
        with (
            tc.tile_pool(name="sb", bufs=1) as sb,
            tc.tile_pool(name="sb2", bufs=2) as sb2,
            tc.tile_pool(name="ps_t", bufs=2, space="PSUM") as ps_t,
            tc.tile_pool(name="ps_q", bufs=1, space="PSUM") as ps_q,
            tc.tile_pool(name="ps_m", bufs=2, space="PSUM") as ps_m,
        ):
            # ---- loads ----
            x_sb = sb.tile([T, D], f32)
            nc.sync.dma_start(out=x_sb[:], in_=x_h[:])

            ident = sb.tile([128, 128], f32)
            make_identity(nc, ident[:])

            # gamma/beta in transposed-feature layout [128, DCH]: partition p of
            # chunk c holds feature c*128+p — matches xlnT / wq_sb layouts.
            gamT = sb.tile([128, D // 128], f32)
            nc.sync.dma_start(out=gamT[:], in_=gam_h[0, :].rearrange("(c p) -> p c", p=128))
            betT = sb.tile([128, D // 128], f32)
            nc.sync.dma_start(out=betT[:], in_=bet_h[0, :].rearrange("(c p) -> p c", p=128))
            bq_sb = sb.tile([1, KD], f32)
            nc.sync.dma_start(out=bq_sb[:], in_=bq_h[:])
            bqc_sb = sb.tile([1, D], f32)
            nc.sync.dma_start(out=bqc_sb[:], in_=bqc_h[:])
            ones_1T = sb.tile([1, T], f32)
            nc.vector.memset(ones_1T[:], 1.0)

            wq_sb = sb.tile([128, DCH, KD], f32)
            nc.sync.dma_start(
                out=wq_sb[:], in_=wq_h[:].rearrange("(c p) n -> p c n", p=128)
            )
            # Wqc split into K-halves so each matmul operand sits at base partition 0
            wqc_a = sb.tile([H, D], f32)
            nc.sync.dma_start(out=wqc_a[:], in_=wqc_h[:H, :])
            wqc_b = sb.tile([H, D], f32)
            nc.sync.dma_start(out=wqc_b[:], in_=wqc_h[H:, :])

            rk_sb = sb.tile([KP, KCH, H], f32)
            nc.sync.dma_start(
                out=rk_sb[:], in_=rk_h[:].rearrange("(c p) k -> p c k", p=KP)
            )
            ck_sb = sb.tile([KP, KCH, H], f32)
            nc.sync.dma_start(
                out=ck_sb[:], in_=ck_h[:].rearrange("(c p) k -> p c k", p=KP)
            )

            # ---- transpose keys: [500,64] -> [64,500] via PE ----
            rkT = sb.tile([H, NK], f32)
            ckT = sb.tile([H, NK], f32)
            for src, dst in ((rk_sb, rkT), (ck_sb, ckT)):
                for c in range(KCH):
                    ktp = ps_t.tile([H, KP], f32, tag="tp")
                    nc.tensor.transpose(
                        out=ktp[:], in_=src[:, c, :], identity=ident[:KP, :KP]
                    )
                    nc.vector.tensor_copy(
                        out=dst[:, c * KP : (c + 1) * KP], in_=ktp[:]
                    )

            # ---- LayerNorm over D=768 (3 bn_stats subgroups of 256) ----
            NSG = 3
            SG = D // NSG
            stats = sb.tile([T, NSG, 6], f32)
            x_view = x_sb[:].rearrange("p (s f) -> p s f", f=SG)
            for s in range(NSG):
                nc.vector.bn_stats(out=stats[:, s, :], in_=x_view[:, s, :])
            mv = sb.tile([T, 2], f32)
            nc.vector.bn_aggr(out=mv[:], in_=stats[:])

            eps_t = sb.tile([T, 1], f32)
            nc.vector.memset(eps_t[:], 1e-5)
            rstd = sb.tile([T, 1], f32)
            # rstd = 1/sqrt(var + eps)
            nc.scalar.activation(
                out=rstd[:],
                in_=mv[:, 1:2],
                func=mybir.ActivationFunctionType.Sqrt,
                bias=eps_t[:],
                scale=1.0,
            )
            nc.vector.reciprocal(out=rstd[:], in_=rstd[:])

            xln = sb.tile([T, D], f32)
            nc.vector.tensor_scalar(
                out=xln[:],
                in0=x_sb[:],
                scalar1=mv[:, 0:1],
                scalar2=rstd[:],
                op0=mybir.AluOpType.subtract,
                op1=mybir.AluOpType.mult,
            )

            # ---- transpose xln chunks -> xlnT [128, 6*T]; gamma folds in as a
            # per-partition scale on the PSUM->SBUF copy ----
            xlnT = sb.tile([128, DCH, T], f32)
            for c in range(DCH):
                tp = ps_t.tile([128, T], f32, tag="tp")
                nc.tensor.transpose(
                    out=tp[:],
                    in_=xln[:, c * 128 : (c + 1) * 128],
                    identity=ident[:T, :T],
                )
                nc.vector.tensor_scalar_mul(xlnT[:, c, :], tp[:], gamT[:, c : c + 1])

            # ---- queries = xln_gamma @ Wq + (beta @ Wq + bq) ----
            # row vector beta @ Wq [1, KD], accumulated on PE
            br_ps = ps_q.tile([1, KD], f32, tag="br")
            for c in range(DCH):
                nc.tensor.matmul(
                    out=br_ps[:],
                    lhsT=betT[:, c : c + 1],
                    rhs=wq_sb[:, c, :],
                    start=(c == 0),
                    stop=(c == DCH - 1),
                )
            qbias = sb.tile([1, KD], f32)
            nc.vector.tensor_add(out=qbias[:], in0=br_ps[:], in1=bq_sb[:])

            q_ps = ps_q.tile([T, KD], f32)
            for c in range(DCH):
                nc.tensor.matmul(
                    out=q_ps[:],
                    lhsT=xlnT[:, c, :],
                    rhs=wq_sb[:, c, :],
                    start=(c == 0),
                    stop=False,
                )
            # broadcast-add the bias row: ones[T,1] @ qbias[1,KD]
            nc.tensor.matmul(
                out=q_ps[:], lhsT=ones_1T[:], rhs=qbias[:], start=False, stop=True
            )
            q_sb = sb.tile([T, KD], f32)
            nc.vector.tensor_copy(out=q_sb[:], in_=q_ps[:])

            # ---- qrT / qcT: transpose each query half -> [64, T] at base 0 ----
            qrT = sb.tile([H, T], f32)
            qcT = sb.tile([H, T], f32)
            for half, dst in enumerate((qrT, qcT)):
                qh_ps = ps_t.tile([H, T], f32, tag="tp")
                nc.tensor.transpose(
                    out=qh_ps[:],
                    in_=q_sb[:, half * H : (half + 1) * H],
                    identity=ident[:T, :T],
                )
                nc.vector.tensor_copy(out=dst[:], in_=qh_ps[:])

            # ---- scores: [T, 500] row & col ----
            rs_sb = sb.tile([T, NK], f32)
            cs_sb = sb.tile([T, NK], f32)
            for qhT, keysT, dst in ((qrT, rkT, rs_sb), (qcT, ckT, cs_sb)):
                sc_ps = ps_m.tile([T, NK], f32, tag="sc")
                nc.tensor.matmul(
                    out=sc_ps[:],
                    lhsT=qhT[:],
                    rhs=keysT[:],
                    start=True,
                    stop=True,
                )
                nc.vector.tensor_copy(out=dst[:], in_=sc_ps[:])

            # ---- top-8 of each, build 16 candidates from top-4 x top-4 ----
            rv8 = sb.tile([T, 8], f32)
            ri8 = sb.tile([T, 8], u32)
            cv8 = sb.tile([T, 8], f32)
            ci8 = sb.tile([T, 8], u32)
            nc.vector.max(out=rv8[:], in_=rs_sb[:])
            nc.vector.max_index(out=ri8[:], in_max=rv8[:], in_values=rs_sb[:])
            nc.vector.max(out=cv8[:], in_=cs_sb[:])
            nc.vector.max_index(out=ci8[:], in_max=cv8[:], in_values=cs_sb[:])

            import concourse.bass as bass_mod

            def bcast_outer(ap):   # [T,4] -> [T,4,4] repeating each elem 4x
                return bass_mod.AP(
                    tensor=ap.tensor, offset=ap.offset,
                    ap=[ap.ap[0], ap.ap[1], [0, TOPK]],
                )

            def bcast_inner(ap):   # [T,4] -> [T,4,4] cycling the 4 elems
                return bass_mod.AP(
                    tensor=ap.tensor, offset=ap.offset,
                    ap=[ap.ap[0], [0, TOPK], ap.ap[1]],
                )

            NC2 = TOPK * TOPK  # 16
            cand = sb.tile([T, NC2], f32)
            cand3 = cand[:].rearrange("p (a b) -> p a b", b=TOPK)
            nc.vector.tensor_tensor(
                out=cand3,
                in0=bcast_outer(rv8[:, :TOPK]),
                in1=bcast_inner(cv8[:, :TOPK]),
                op=mybir.AluOpType.add,
            )

            riF = sb.tile([T, TOPK], f32)
            ciF = sb.tile([T, TOPK], f32)
            nc.vector.tensor_copy(out=riF[:], in_=ri8[:, :TOPK])
            nc.vector.tensor_copy(out=ciF[:], in_=ci8[:, :TOPK])
            nc.vector.tensor_scalar_mul(riF[:], riF[:], float(NK))
            flatc = sb.tile([T, NC2], f32)
            flatc3 = flatc[:].rearrange("p (a b) -> p a b", b=TOPK)
            nc.vector.tensor_tensor(
                out=flatc3,
                in0=bcast_outer(riF[:]),
                in1=bcast_inner(ciF[:]),
                op=mybir.AluOpType.add,
            )

            cm8 = sb.tile([T, 8], f32)
            nc.vector.max(out=cm8[:], in_=cand[:])
            nc.sync.dma_start(out=os_h[:], in_=cm8[:, :TOPK])

            # select flat idx of each of the top-4 candidate values
            fsel = sb.tile([T, TOPK], f32)
            for k in range(TOPK):
                mask = sb2.tile([T, NC2], f32, tag="mask")
                nc.vector.tensor_scalar(
                    out=mask[:],
                    in0=cand[:],
                    scalar1=cm8[:, k : k + 1],
                    scalar2=None,
                    op0=mybir.AluOpType.is_equal,
                )
                masked = sb2.tile([T, NC2], f32, tag="masked")
                nc.vector.tensor_mul(out=masked[:], in0=mask[:], in1=flatc[:])
                junk = sb2.tile([T, NC2], f32, tag="junk")
                # reduce on ScalarE: accum_out = sum over free dim
                nc.scalar.activation(
                    out=junk[:],
                    in_=masked[:],
                    func=mybir.ActivationFunctionType.Copy,
                    bias=0.0,
                    scale=1.0,
                    accum_out=fsel[:, k : k + 1],
                )

            idx_i = sb.tile([T, TOPK], i32)
            nc.vector.tensor_copy(out=idx_i[:], in_=fsel[:])
            nc.sync.dma_start(out=oi_h[:], in_=idx_i[:])

            # ---- gather concepts[idx] : 4 indirect DMAs of 32 rows each ----
            gath = sb.tile([T, TOPK, D], f32)
            for k in range(TOPK):
                nc.gpsimd.indirect_dma_start(
                    out=gath[:, k, :],
                    out_offset=None,
                    in_=cpt_h[:],
                    in_offset=bass_mod.IndirectOffsetOnAxis(
                        ap=idx_i[:, k : k + 1], axis=0
                    ),
                )
            nc.sync.dma_start(
                out=oc_h[:].rearrange("(t k) d -> t k d", k=TOPK), in_=gath[:]
            )

            # ---- query_projected = queries @ Wqc + bqc ----
            # contract K=128 as two half-K accumulating matmuls (operands at base 0)
            qp_sb = sb.tile([T, D], f32)
            NSPL = 2
            W = D // NSPL  # 384
            for j in range(NSPL):
                qp_ps = ps_m.tile([T, W], f32, tag="qp")
                for half, (qhT, wqc_half) in enumerate(((qrT, wqc_a), (qcT, wqc_b))):
                    nc.tensor.matmul(
                        out=qp_ps[:],
                        lhsT=qhT[:],
                        rhs=wqc_half[:, j * W : (j + 1) * W],
                        start=(half == 0),
                        stop=False,
                    )
                nc.tensor.matmul(
                    out=qp_ps[:],
                    lhsT=ones_1T[:],
                    rhs=bqc_sb[:, j * W : (j + 1) * W],
                    start=False,
                    stop=True,
                )
                nc.vector.tensor_copy(
                    out=qp_sb[:, j * W : (j + 1) * W], in_=qp_ps[:]
                )
            nc.sync.dma_start(out=oq_h[:], in_=qp_sb[:])

    nc.finalize()  # runs Bacc.compile(): reg alloc + sync-wait legalization
    return nc


def _prep_inputs(inputs):
    def f(x):
        return np.ascontiguousarray(np.asarray(x, dtype=np.float32))

    reg = f(inputs["registers"]).reshape(NTOK, D)
    shared = {
        "row_keys": f(inputs["row_keys"]),
        "col_keys": f(inputs["col_keys"]),
        "concepts": f(inputs["concepts"]),
        "ln_gamma": f(inputs["ln_gamma"]).reshape(1, D),
        "ln_beta": f(inputs["ln_beta"]).reshape(1, D),
        "Wq": f(inputs["Wq"]),
        "bq": f(inputs["bq"]).reshape(1, KD),
        "Wqc": f(inputs["Wqc"]),
        "bqc": f(inputs["bqc"]).reshape(1, D),
    }
    in_maps = []
    for c in range(NCORES):
        m = dict(shared)
        m["x_local"] = np.ascontiguousarray(reg[c * T : (c + 1) * T])
        in_maps.append(m)
    return in_maps


def _assemble(results):
    oc = np.zeros((B, R * TOPK, D), dtype=np.float32)
    oi = np.zeros((B, R, TOPK), dtype=np.int32)
    osc = np.zeros((B, R, TOPK), dtype=np.float32)
    oq = np.zeros((B, R, D), dtype=np.float32)
    per_b = R // T  # cores per batch element
    for c in range(NCORES):
        b, s = divmod(c, per_b)
        r0 = s * T
        oc[b, r0 * TOPK : (r0 + T) * TOPK] = results[c]["out_concepts"]
        oi[b, r0 : r0 + T] = results[c]["out_idx"]
        osc[b, r0 : r0 + T] = results[c]["out_scores"]
        oq[b, r0 : r0 + T] = results[c]["out_qproj"]
    return oc, oi, osc, oq


def kernel(**inputs):
    from concourse.bass_utils import run_bass_kernel_spmd

    if "nc" not in _CACHE:
        _CACHE["nc"] = _build_nc()
    nc = _CACHE["nc"]
    in_maps = _prep_inputs(inputs)
    res = run_bass_kernel_spmd(nc, in_maps, core_ids=list(range(NCORES)))
    _CACHE["last_results"] = res
    return _assemble(res.results)
